# revision 14
# baseline (speedup 1.0000x reference)
"""Conv2d 3x3 (stride 1, pad 1) + bias on Trainium2, data-parallel over batch.

Full problem: x [32,128,56,56] f32, filters [256,128,3,3], biases [256]
-> out [32,256,56,56].  8 NeuronCores, 4 images per core.

Per-core kernel: 1D Winograd F(2,3) along W cuts PE work 1.5x vs the
direct 9-tap formulation (3 matmuls -> 2 effective per output column):
  - host precomputes the input transform X~[b,c,h,p,t] (p in 0..3 tap
    positions, t in 0..27 tiles of 2 output cols) and the weight
    transform W~[p,dy] = G @ w[dy,:] per dy row of the 3x3 kernel.
  - device: m_p = sum_dy W~[p,dy]^T @ X~[rows r+dy, p]  (PSUM, fp32),
    i.e. 12 matmuls of 392 moving cols per 14-row group instead of
    9 matmuls of 448 per 8-row group.  fp16 operands as before.
  - output transform on DVE with the bias folded into the first op:
      y_even = m0 + (m1 + bias + m2)      y_odd = (m1 + bias - m2) - m3
    via scalar_tensor_tensor (2 ops each), writing w-interleaved into
    the output SBUF tile so the store DMA stays fully contiguous.
X~ is loaded in 16-row halo chunks (one per 14-row output group) so the
first matmuls start after ~0.5 MB of DMA.  Output DMAs alternate between
the DVE and ACT HWDGE queues (inputs ride SP).
"""

import numpy as np

import concourse.bass as bass
import concourse.mybir as mybir
import concourse.tile as tile
from concourse import bacc
from concourse import bass_utils as _bass_utils
from concourse.bass_utils import run_bass_kernel_spmd

def _strip_redundant_ldweights(m):
    """Delete InstLdweights whose weights AP matches the previous load with
    only matmuls in between on the tensor engine: the PE array retains its
    stationary weights across matmuls, so the reload is pure overhead
    (~100ns of serial PE time each).  Only sync-free loads are removed so
    no semaphore waits/updates are lost.  With the g-inner loop order below
    this collapses 4 identical consecutive loads into 1."""
    transparent = {"InstMatmult", "InstEventSemaphore", "InstDrain",
                   "InstNop", "InstNotify"}
    n_removed = 0
    for f in m.functions:
        for blk in f.blocks:
            keep = []
            last_key = None
            pending_sync = []  # waits from deleted LDs -> next matmul
            for inst in blk.instructions:
                tn = type(inst).__name__
                if str(inst.engine) != "EngineType.PE":
                    keep.append(inst)
                    continue
                if tn == "InstLdweights":
                    a = inst.ins[0]
                    key = (a.memref, str(a.offset), str(a.ap))
                    si = inst.sync_info
                    if key == last_key and (si is None or not si.on_update):
                        if si is not None and si.on_wait:
                            pending_sync.extend(si.on_wait)
                        n_removed += 1
                        continue
                    last_key = key
                elif tn == "InstMatmult":
                    if pending_sync:
                        si = inst.sync_info
                        waits = list(si.on_wait) if si else []
                        upds = list(si.on_update) if si else []
                        inst.sync_info = mybir.SyncInfo(
                            on_wait=waits + pending_sync, on_update=upds)
                        pending_sync = []
                elif tn not in transparent:
                    # this PE instruction may clobber array state
                    last_key = None
                keep.append(inst)
            assert not pending_sync
            if len(keep) != len(blk.instructions):
                blk.instructions[:] = keep
    return n_removed

NCORES = 8
B, CIN, H, W = 32, 128, 56, 56
COUT, F = 256, 3
BLOC = B // NCORES  # 4 images per core
HP = H + 2          # 58 padded rows
NT = W // 2         # 28 winograd tiles per row
NPOS = 4            # winograd positions per tile
XROW = NPOS * NT    # 112 transformed cols per row
RG = 14             # output rows per matmul group
NGRP = H // RG      # 4 row groups
CROWS = RG + 2      # 16 input rows per chunk (2-row halo)
NMOV = RG * NT      # 392 moving elements per matmul (<=512 PSUM bank)
PSEQ = (1, 2, 0, 3)  # matmul p order: m1,m2 first so DVE starts early

F32 = mybir.dt.float32
F16 = mybir.dt.float16
ADD = mybir.AluOpType.add
SUB = mybir.AluOpType.subtract

_CACHE = {}


def _build_nc():
    nc = bacc.Bacc("TRN2", target_bir_lowering=False, debug=False,
                   num_devices=NCORES)
    xw_d = nc.dram_tensor("xw", [BLOC, CIN, HP, XROW], F16,
                          kind="ExternalInput").ap()
    wt_d = nc.dram_tensor("wt", [CIN, 2 * 4 * 3 * 128], F16,
                          kind="ExternalInput").ap()
    bias_d = nc.dram_tensor("bias", [128, 2], F32, kind="ExternalInput").ap()
    out_d = nc.dram_tensor("out", [BLOC, COUT, H, W], F32,
                           kind="ExternalOutput").ap()

    with tile.TileContext(nc) as tc:
        with (
            tc.tile_pool(name="weights", bufs=1) as wpool,
            tc.tile_pool(name="xin", bufs=1) as xpool,
            tc.tile_pool(name="outs", bufs=6) as opool,
            tc.tile_pool(name="scr", bufs=4) as spool,
            tc.tile_pool(name="psum", bufs=8, space="PSUM") as ppool,
        ):
            # PE warm-up: the HAM clock gate keeps the PE at 1.2 GHz until
            # it has seen ~3.4us of sustained activity.  Burn that window on
            # dummy matmuls over a zeroed tile while the input DMAs stream.
            # Warm operands come from the preamble-written const tiles so
            # the first warm matmul needs no memset or DMA to complete --
            # it issues as soon as the PE preamble ends (fp32 runs slower
            # on the PE, which is fine: HAM only counts busy time).
            wlhs = nc.const_aps.tensor(1.0, [128, 128], F32)
            wrhs = nc.const_aps.tensor(1.0, [128, NMOV], F32)
            wps = ppool.tile([128, NMOV], F32, name="wps", tag="ps")
            for _ in range(8):
                nc.tensor.matmul(wps[:], wlhs, wrhs, start=True, stop=True)

            xtiles = {}

            def load_chunk(b, g):
                r0 = g * RG
                xt = xpool.tile([CIN, CROWS * XROW], F16, name=f"x{b}g{g}")
                nc.sync.dma_start(
                    xt[:],
                    xw_d[b, :, r0:r0 + CROWS, :].rearrange("c h w -> c (h w)"))
                xtiles[(b, g)] = xt

            # Weights stream per (half, p) block of 3*128 cout cols so the
            # first matmul gates on one 96 KB block + the first x chunk.
            wt_sb = wpool.tile([CIN, 2 * 4 * 3 * 128], F16, name="wt_sb")
            nc.scalar.dma_start(wt_sb[:, 0:384], wt_d[:, 0:384])
            load_chunk(0, 0)
            bias_sb = wpool.tile([128, 2], F32, name="bias_sb")
            nc.scalar.dma_start(bias_sb[:], bias_d[:])
            for blk in range(1, 4):
                nc.scalar.dma_start(wt_sb[:, blk * 384:(blk + 1) * 384],
                                    wt_d[:, blk * 384:(blk + 1) * 384])
            load_chunk(0, 1)
            for blk in range(4, 8):
                nc.sync.dma_start(wt_sb[:, blk * 384:(blk + 1) * 384],
                                  wt_d[:, blk * 384:(blk + 1) * 384])
            load_chunk(0, 2)
            load_chunk(0, 3)
            for b in range(1, BLOC):
                for g in range(NGRP):
                    load_chunk(b, g)

            # Per (g,half) iteration: 12 matmuls accumulate m0..m3 in 4
            # PSUM banks (bufs=8 double-buffers across iterations).  The
            # output transform is split across three engines:
            #   ACT:    e1 = m1+bias (PSUM->SBUF),  a3 = copy(m3)
            #   DVE:    d1 = e1+m2, d2 = e1-m2, y_even = d1+m0
            #   GPSIMD: y_odd = d2-a3   (SBUF-only; GPSIMD has no PSUM port)
            # Every DVE op pairs one PSUM operand with one SBUF operand (a
            # DVE op may read at most one PSUM tensor).
            ndma = 0
            for b in range(BLOC):
                for g in range(NGRP):
                    xv = xtiles[(b, g)][:].rearrange(
                        "c (h p t) -> c h p t", h=CROWS, p=NPOS)
                    for half in range(2):
                        ms = {}
                        for pi, p in enumerate(PSEQ):
                            mt = ppool.tile([128, NMOV], F32, name=f"m{p}",
                                            tag="ps")
                            for dy in range(F):
                                c0 = ((half * 4 + pi) * 3 + dy) * 128
                                nc.tensor.matmul(
                                    mt[:], wt_sb[:, c0:c0 + 128],
                                    xv[:, dy:dy + RG, p:p + 1, :],
                                    start=(dy == 0), stop=(dy == F - 1))
                            ms[p] = mt
                            if p == 1:
                                e1 = spool.tile([128, NMOV], F32, name="e1")
                                nc.scalar.add(e1[:], mt[:],
                                              bias_sb[:, half:half + 1])
                        ot = opool.tile([128, RG * W], F32, name="ot")
                        otv = ot[:].rearrange("c (h t j) -> c h t j",
                                              h=RG, t=NT, j=2)
                        d1 = spool.tile([128, NMOV], F32, name="d1")
                        d2 = spool.tile([128, NMOV], F32, name="d2")
                        a3 = spool.tile([128, NMOV], F32, name="a3")
                        nc.vector.tensor_tensor(d1[:], e1[:], ms[2][:], ADD)
                        nc.vector.tensor_tensor(d2[:], e1[:], ms[2][:], SUB)
                        nc.scalar.copy(a3[:], ms[3][:])
                        m0v = ms[0][:].rearrange("c (h t) -> c h t",
                                                 h=RG).unsqueeze(3)
                        d1v = d1[:].rearrange("c (h t) -> c h t",
                                              h=RG).unsqueeze(3)
                        d2v = d2[:].rearrange("c (h t) -> c h t",
                                              h=RG).unsqueeze(3)
                        a3v = a3[:].rearrange("c (h t) -> c h t",
                                              h=RG).unsqueeze(3)
                        nc.vector.tensor_tensor(
                            otv[:, :, :, 0:1], d1v, m0v, ADD)
                        nc.gpsimd.tensor_tensor(
                            otv[:, :, :, 1:2], d2v, a3v, SUB)
                        dst = out_d[b, half * 128:half * 128 + 128,
                                    g * RG:(g + 1) * RG, :]
                        eng = (nc.scalar, nc.sync,
                               nc.gpsimd)[ndma % 3]
                        ndma += 1
                        eng.dma_start(
                            dst.rearrange("o h w -> o (h w)"), ot[:])
    _strip_redundant_ldweights(nc.m)
    nc.compile()
    return nc


def _get_nc():
    if "nc" not in _CACHE:
        _CACHE["nc"] = _build_nc()
    return _CACHE["nc"]


def _prep(x, filters, biases):
    # Input transform: pad, then 1D F(2,3) along W.
    xp = np.zeros((B, CIN, HP, HP), np.float32)
    xp[:, :, 1:1 + H, 1:1 + W] = x
    d0 = xp[..., 0:56:2]
    d1 = xp[..., 1:57:2]
    d2 = xp[..., 2:58:2]
    d3 = xp[..., 3:58:2]
    xw = np.empty((B, CIN, HP, NPOS, NT), np.float32)
    xw[..., 0, :] = d0 - d2
    xw[..., 1, :] = d1 + d2
    xw[..., 2, :] = d2 - d1
    xw[..., 3, :] = d1 - d3
    xw16 = xw.astype(np.float16).reshape(B, CIN, HP, XROW)

    # Weight transform: W~[p,dy] = G @ w[dy,:] per cin/cout.
    G = np.array([[1, 0, 0], [.5, .5, .5], [.5, -.5, .5], [0, 0, 1]],
                 np.float32)
    wt_all = np.einsum("pj,oidj->pdio", G, filters.astype(np.float32))
    wt_perm = wt_all[list(PSEQ)]                    # [4p, 3dy, 128i, 256o]
    wt_r = wt_perm.reshape(4, 3, CIN, 2, 128)       # o -> (half, ol)
    wt = np.ascontiguousarray(
        wt_r.transpose(2, 3, 0, 1, 4)).reshape(CIN, 3072).astype(np.float16)

    bias2 = np.ascontiguousarray(biases.reshape(2, 128).T.astype(np.float32))
    return xw16, wt, bias2


def make_in_maps(x, filters, biases):
    xw16, wt, bias2 = _prep(x, filters, biases)
    return [
        {"xw": xw16[c * BLOC:(c + 1) * BLOC], "wt": wt, "bias": bias2}
        for c in range(NCORES)
    ]


def kernel(x, filters, biases):
    x = np.ascontiguousarray(x, dtype=np.float32)
    filters = np.ascontiguousarray(filters, dtype=np.float32)
    biases = np.ascontiguousarray(biases, dtype=np.float32)

    nc = _get_nc()
    in_maps = make_in_maps(x, filters, biases)
    res = run_bass_kernel_spmd(nc, in_maps, list(range(NCORES)))
    out = np.concatenate([res.results[c]["out"] for c in range(NCORES)],
                         axis=0)
    return out


# revision 15
# speedup vs baseline: 1.0255x; 1.0255x over previous
"""Conv2d 3x3 (stride 1, pad 1) + bias on Trainium2, data-parallel over batch.

Full problem: x [32,128,56,56] f32, filters [256,128,3,3], biases [256]
-> out [32,256,56,56].  8 NeuronCores, 4 images per core.

Per-core kernel: 1D Winograd F(2,3) along W cuts PE work 1.5x vs the
direct 9-tap formulation (3 matmuls -> 2 effective per output column):
  - host precomputes the input transform X~[b,c,h,p,t] (p in 0..3 tap
    positions, t in 0..27 tiles of 2 output cols) and the weight
    transform W~[p,dy] = G @ w[dy,:] per dy row of the 3x3 kernel.
  - device: m_p = sum_dy W~[p,dy]^T @ X~[rows r+dy, p]  (PSUM, fp32),
    i.e. 12 matmuls of 392 moving cols per 14-row group instead of
    9 matmuls of 448 per 8-row group.  fp16 operands as before.
  - output transform on DVE with the bias folded into the first op:
      y_even = m0 + (m1 + bias + m2)      y_odd = (m1 + bias - m2) - m3
    via scalar_tensor_tensor (2 ops each), writing w-interleaved into
    the output SBUF tile so the store DMA stays fully contiguous.
X~ is loaded in 16-row halo chunks (one per 14-row output group) so the
first matmuls start after ~0.5 MB of DMA.  Output DMAs alternate between
the DVE and ACT HWDGE queues (inputs ride SP).
"""

import numpy as np

import concourse.bass as bass
import concourse.mybir as mybir
import concourse.tile as tile
from concourse import bacc
from concourse import bass_utils as _bass_utils
from concourse.bass_utils import run_bass_kernel_spmd

def _strip_redundant_ldweights(m):
    """Delete InstLdweights whose weights AP matches the previous load with
    only matmuls in between on the tensor engine: the PE array retains its
    stationary weights across matmuls, so the reload is pure overhead
    (~100ns of serial PE time each).  Only sync-free loads are removed so
    no semaphore waits/updates are lost.  With the g-inner loop order below
    this collapses 4 identical consecutive loads into 1."""
    transparent = {"InstMatmult", "InstEventSemaphore", "InstDrain",
                   "InstNop", "InstNotify"}
    n_removed = 0
    for f in m.functions:
        for blk in f.blocks:
            keep = []
            last_key = None
            pending_sync = []  # waits from deleted LDs -> next matmul
            for inst in blk.instructions:
                tn = type(inst).__name__
                if str(inst.engine) != "EngineType.PE":
                    keep.append(inst)
                    continue
                if tn == "InstLdweights":
                    a = inst.ins[0]
                    key = (a.memref, str(a.offset), str(a.ap))
                    si = inst.sync_info
                    if key == last_key and (si is None or not si.on_update):
                        if si is not None and si.on_wait:
                            pending_sync.extend(si.on_wait)
                        n_removed += 1
                        continue
                    last_key = key
                elif tn == "InstMatmult":
                    if pending_sync:
                        si = inst.sync_info
                        waits = list(si.on_wait) if si else []
                        upds = list(si.on_update) if si else []
                        inst.sync_info = mybir.SyncInfo(
                            on_wait=waits + pending_sync, on_update=upds)
                        pending_sync = []
                elif tn not in transparent:
                    # this PE instruction may clobber array state
                    last_key = None
                keep.append(inst)
            assert not pending_sync
            if len(keep) != len(blk.instructions):
                blk.instructions[:] = keep
    return n_removed

NCORES = 8
B, CIN, H, W = 32, 128, 56, 56
COUT, F = 256, 3
BLOC = B // NCORES  # 4 images per core
HP = H + 2          # 58 padded rows
NT = W // 2         # 28 winograd tiles per row
NPOS = 4            # winograd positions per tile
XROW = NPOS * NT    # 112 transformed cols per row
RG = 14             # output rows per matmul group
NGRP = H // RG      # 4 row groups
CROWS = RG + 2      # 16 input rows per chunk (2-row halo)
NMOV = RG * NT      # 392 moving elements per matmul (<=512 PSUM bank)
PSEQ = (1, 2, 0, 3)  # matmul p order: m1,m2 first so DVE starts early

F32 = mybir.dt.float32
F16 = mybir.dt.float16
ADD = mybir.AluOpType.add
SUB = mybir.AluOpType.subtract

_CACHE = {}


def _build_nc():
    nc = bacc.Bacc("TRN2", target_bir_lowering=False, debug=False,
                   num_devices=NCORES)
    xw_d = nc.dram_tensor("xw", [BLOC, CIN, HP, XROW], F16,
                          kind="ExternalInput").ap()
    wt_d = nc.dram_tensor("wt", [CIN, 2 * 4 * 3 * 128], F16,
                          kind="ExternalInput").ap()
    bias_d = nc.dram_tensor("bias", [128, 2], F32, kind="ExternalInput").ap()
    out_d = nc.dram_tensor("out", [BLOC, COUT, H, W], F32,
                           kind="ExternalOutput").ap()

    with tile.TileContext(nc) as tc:
        with (
            tc.tile_pool(name="weights", bufs=1) as wpool,
            tc.tile_pool(name="xin", bufs=1) as xpool,
            tc.tile_pool(name="outs", bufs=6) as opool,
            tc.tile_pool(name="scr", bufs=4) as spool,
            tc.tile_pool(name="psum", bufs=8, space="PSUM") as ppool,
        ):
            # PE warm-up: the HAM clock gate keeps the PE at 1.2 GHz until
            # it has seen ~3.4us of sustained activity.  Burn that window on
            # dummy matmuls over a zeroed tile while the input DMAs stream.
            # Warm operands come from the preamble-written bf16 const tile
            # so the first warm matmul needs no memset or DMA to complete --
            # it issues as soon as the PE preamble ends and runs at the
            # full 1 col/cycle 16-bit rate.
            BF16 = mybir.dt.bfloat16
            wlhs = nc.const_aps.tensor(1.0, [128, 128], BF16)
            wrhs = nc.const_aps.tensor(1.0, [128, NMOV], BF16)
            wps = ppool.tile([128, NMOV], F32, name="wps", tag="ps")
            for _ in range(14):
                nc.tensor.matmul(wps[:], wlhs, wrhs, start=True, stop=True)

            xtiles = {}

            def load_chunk(b, g):
                r0 = g * RG
                xt = xpool.tile([CIN, CROWS * XROW], F16, name=f"x{b}g{g}")
                nc.sync.dma_start(
                    xt[:],
                    xw_d[b, :, r0:r0 + CROWS, :].rearrange("c h w -> c (h w)"))
                xtiles[(b, g)] = xt

            # Weights stream per (half, p) block of 3*128 cout cols so the
            # first matmul gates on one 96 KB block + the first x chunk.
            wt_sb = wpool.tile([CIN, 2 * 4 * 3 * 128], F16, name="wt_sb")
            nc.scalar.dma_start(wt_sb[:, 0:384], wt_d[:, 0:384])
            load_chunk(0, 0)
            bias_sb = wpool.tile([128, 2], F32, name="bias_sb")
            nc.scalar.dma_start(bias_sb[:], bias_d[:])
            for blk in range(1, 4):
                nc.scalar.dma_start(wt_sb[:, blk * 384:(blk + 1) * 384],
                                    wt_d[:, blk * 384:(blk + 1) * 384])
            load_chunk(0, 1)
            for blk in range(4, 8):
                nc.sync.dma_start(wt_sb[:, blk * 384:(blk + 1) * 384],
                                  wt_d[:, blk * 384:(blk + 1) * 384])
            load_chunk(0, 2)
            load_chunk(0, 3)
            for b in range(1, BLOC):
                for g in range(NGRP):
                    load_chunk(b, g)

            # Per (g,half) iteration: 12 matmuls accumulate m0..m3 in 4
            # PSUM banks (bufs=8 double-buffers across iterations).  The
            # output transform is split across three engines:
            #   ACT:    e1 = m1+bias (PSUM->SBUF),  a3 = copy(m3)
            #   DVE:    d1 = e1+m2, d2 = e1-m2, y_even = d1+m0
            #   GPSIMD: y_odd = d2-a3   (SBUF-only; GPSIMD has no PSUM port)
            # Every DVE op pairs one PSUM operand with one SBUF operand (a
            # DVE op may read at most one PSUM tensor).
            ndma = 0
            for b in range(BLOC):
                for g in range(NGRP):
                    xv = xtiles[(b, g)][:].rearrange(
                        "c (h p t) -> c h p t", h=CROWS, p=NPOS)
                    for half in range(2):
                        ms = {}
                        for pi, p in enumerate(PSEQ):
                            mt = ppool.tile([128, NMOV], F32, name=f"m{p}",
                                            tag="ps")
                            for dy in range(F):
                                c0 = ((half * 4 + pi) * 3 + dy) * 128
                                nc.tensor.matmul(
                                    mt[:], wt_sb[:, c0:c0 + 128],
                                    xv[:, dy:dy + RG, p:p + 1, :],
                                    start=(dy == 0), stop=(dy == F - 1))
                            ms[p] = mt
                            if p == 1:
                                e1 = spool.tile([128, NMOV], F32, name="e1")
                                nc.scalar.add(e1[:], mt[:],
                                              bias_sb[:, half:half + 1])
                        ot = opool.tile([128, RG * W], F32, name="ot")
                        otv = ot[:].rearrange("c (h t j) -> c h t j",
                                              h=RG, t=NT, j=2)
                        d1 = spool.tile([128, NMOV], F32, name="d1")
                        d2 = spool.tile([128, NMOV], F32, name="d2")
                        a3 = spool.tile([128, NMOV], F32, name="a3")
                        nc.vector.tensor_tensor(d1[:], e1[:], ms[2][:], ADD)
                        nc.vector.tensor_tensor(d2[:], e1[:], ms[2][:], SUB)
                        nc.scalar.copy(a3[:], ms[3][:])
                        m0v = ms[0][:].rearrange("c (h t) -> c h t",
                                                 h=RG).unsqueeze(3)
                        d1v = d1[:].rearrange("c (h t) -> c h t",
                                              h=RG).unsqueeze(3)
                        d2v = d2[:].rearrange("c (h t) -> c h t",
                                              h=RG).unsqueeze(3)
                        a3v = a3[:].rearrange("c (h t) -> c h t",
                                              h=RG).unsqueeze(3)
                        nc.vector.tensor_tensor(
                            otv[:, :, :, 0:1], d1v, m0v, ADD)
                        nc.gpsimd.tensor_tensor(
                            otv[:, :, :, 1:2], d2v, a3v, SUB)
                        dst = out_d[b, half * 128:half * 128 + 128,
                                    g * RG:(g + 1) * RG, :]
                        eng = (nc.sync, nc.scalar,
                               nc.sync)[ndma % 3]
                        ndma += 1
                        eng.dma_start(
                            dst.rearrange("o h w -> o (h w)"), ot[:])
    _strip_redundant_ldweights(nc.m)
    nc.compile()
    return nc


def _get_nc():
    if "nc" not in _CACHE:
        _CACHE["nc"] = _build_nc()
    return _CACHE["nc"]


def _prep(x, filters, biases):
    # Input transform: pad, then 1D F(2,3) along W.
    xp = np.zeros((B, CIN, HP, HP), np.float32)
    xp[:, :, 1:1 + H, 1:1 + W] = x
    d0 = xp[..., 0:56:2]
    d1 = xp[..., 1:57:2]
    d2 = xp[..., 2:58:2]
    d3 = xp[..., 3:58:2]
    xw = np.empty((B, CIN, HP, NPOS, NT), np.float32)
    xw[..., 0, :] = d0 - d2
    xw[..., 1, :] = d1 + d2
    xw[..., 2, :] = d2 - d1
    xw[..., 3, :] = d1 - d3
    xw16 = xw.astype(np.float16).reshape(B, CIN, HP, XROW)

    # Weight transform: W~[p,dy] = G @ w[dy,:] per cin/cout.
    G = np.array([[1, 0, 0], [.5, .5, .5], [.5, -.5, .5], [0, 0, 1]],
                 np.float32)
    wt_all = np.einsum("pj,oidj->pdio", G, filters.astype(np.float32))
    wt_perm = wt_all[list(PSEQ)]                    # [4p, 3dy, 128i, 256o]
    wt_r = wt_perm.reshape(4, 3, CIN, 2, 128)       # o -> (half, ol)
    wt = np.ascontiguousarray(
        wt_r.transpose(2, 3, 0, 1, 4)).reshape(CIN, 3072).astype(np.float16)

    bias2 = np.ascontiguousarray(biases.reshape(2, 128).T.astype(np.float32))
    return xw16, wt, bias2


def make_in_maps(x, filters, biases):
    xw16, wt, bias2 = _prep(x, filters, biases)
    return [
        {"xw": xw16[c * BLOC:(c + 1) * BLOC], "wt": wt, "bias": bias2}
        for c in range(NCORES)
    ]


def kernel(x, filters, biases):
    x = np.ascontiguousarray(x, dtype=np.float32)
    filters = np.ascontiguousarray(filters, dtype=np.float32)
    biases = np.ascontiguousarray(biases, dtype=np.float32)

    nc = _get_nc()
    in_maps = make_in_maps(x, filters, biases)
    res = run_bass_kernel_spmd(nc, in_maps, list(range(NCORES)))
    out = np.concatenate([res.results[c]["out"] for c in range(NCORES)],
                         axis=0)
    return out
